# revision 12
# baseline (speedup 1.0000x reference)
"""Trainium2 Bass kernel for HardConstrainedMLP (MLP + 100-iter dual projected
gradient projection onto {y : Ay <= b}).

Math rewrite (verified to 4.6e-7 vs reference):
    y0 = MLP(x)
    t  = 1/||A||_F^2 ; G = A@A.T ; H = I - t*G ; c = t*(y0@A.T - b)
    lam_{i+1} = relu(lam_i @ H + c)        (100 iters, lam_0 = 0)
    y = y0 - lam_100 @ A

On-device layout is transposed (feature-major) so the per-iteration matmul
chain needs no transposes.  Matmuls use float32r (full PE rate at >=256
moving columns).

Inner loop (variant A, default): per iteration two PSUM tiles accumulate
eye@c + H-block matmuls (6 matmuls, 512-wide moving operands = full f32r
rate); relu consumers run unsplit on ACT / DVE (one semaphore per lam
half -- splitting them measurably loses to PE-queue wait-eval cost), and
the next iteration's eye-matmuls are issued at the end of each iteration
so the loop boundary carries no extra stall.  Measured ~1760 ns/iter,
which matches the order-LP floor for this shape (psum-fire -> relu ->
consume gap of ~800 ns against 6x236 ns of PE work).

Variant C (KERNEL_VARIANT=C) keeps a custom fused DVE op relu(psum + c)
with 4 matmuls/iter; measured slower (DVE-bound at ~1880 ns/iter).

Data-parallel over batch: 4096 rows -> 8 cores x 512 rows.
"""

import os
import sys

sys.path.insert(0, "/opt/trn_rl_repo")

import numpy as np

B, IN_DIM, HID, DIM, M = 4096, 256, 200, 512, 256
NCORES = 8
R = B // NCORES  # rows per core

VARIANT = os.environ.get("KERNEL_VARIANT", "A")  # "A" eye-mm | "C" fused-dve
TRACE = False  # set by test.py before calling kernel() to profile the run
LAST = {}      # {"exec_time_ns": int, "trace": path} after a TRACE run

_BUILD_CACHE = {}
_RELU_ADD = None


def _get_relu_add_op():
    """Register (once) a custom DVE op computing out = relu(in0 + in1).

    Uses the stock custom-DVE plumbing in concourse.dve_ops: we assign the
    next free byte-36 opcode row and append to OPS so the per-NEFF table
    generation picks it up."""
    global _RELU_ADD
    if _RELU_ADD is not None:
        return _RELU_ADD
    import concourse.dve_ops as dops
    from concourse.dve_spec import Spec, Src0, Src1, lower, relu, _has_src1
    from concourse.dve_table_gen import dve_ver_for
    from concourse.dve_uop import DveOpSpec

    name = "RELU_ADD_ANT"

    def _ref(in0, in1, s0, s1, imm2):
        x = in0.astype(np.float32) + in1.astype(np.float32)
        x = np.nan_to_num(x, nan=0.0, posinf=np.inf, neginf=-np.inf)
        return np.maximum(x, 0).astype(np.float32)

    spec = Spec(body=relu(Src0 + Src1), reference=_ref)
    if name not in dops._SUB_OPCODE_FOR_NAME:
        row = max(dops._SUB_OPCODE_FOR_NAME.values()) + 1
        assert row < 0x20, "custom-DVE opcode rows exhausted"
        dops._SUB_OPCODE_FOR_NAME[name] = row
        ver = dve_ver_for("TRN2")
        uops = lower(spec, ver=ver)
        sha = DveOpSpec(name=name, opcode=row, uops=uops,
                        rd1_en=_has_src1(spec)).sha(ver)
        op = dops.DveOp(name, spec, subdim=False, uops_sha={ver: sha})
        dops.OPS.append(op)
        dops.CUSTOM_DVE_SPECS[name] = spec
    else:
        op = next(o for o in dops.OPS if o.name == name)
    _RELU_ADD = op
    return op


def _build(n_iter: int):
    import concourse.mybir as mybir
    import concourse.tile as tile
    from concourse import bacc

    F32 = mybir.dt.float32
    F32R = mybir.dt.float32r
    AF = mybir.ActivationFunctionType
    OP = mybir.AluOpType

    relu_add = _get_relu_add_op() if VARIANT == "C" else None

    nc = bacc.Bacc("TRN2", target_bir_lowering=False, debug=False,
                   num_devices=NCORES)

    # ---- per-core inputs (f32r dram = raw fp32 bytes used as matmul operands)
    # All large tensors are host-packed to 128 partitions so each is a single
    # dram tensor; DMAs are column-chunked for transfer parallelism.
    xT_d = nc.dram_tensor("xTp", [128, 2 * R], F32R, kind="ExternalInput")
    bts_d = nc.dram_tensor("btsp", [128, 2 * R], F32, kind="ExternalInput")
    # ---- replicated weights / constants (packed)
    w1_d = nc.dram_tensor("w1p", [128, 2 * HID], F32R, kind="ExternalInput")
    w2_d = nc.dram_tensor("w2p", [128, 2 * HID], F32R, kind="ExternalInput")
    w3_d = nc.dram_tensor("w3p", [128, 2 * DIM], F32R, kind="ExternalInput")
    at_d = nc.dram_tensor("atp", [128, 4 * M], F32R, kind="ExternalInput")
    na_d = nc.dram_tensor("nap", [128, 2 * DIM], F32R, kind="ExternalInput")
    h_d = nc.dram_tensor("hmp", [128, 2 * M], F32R, kind="ExternalInput")
    eye_d = nc.dram_tensor("eye", [128, 128], F32R, kind="ExternalInput")
    # row constants on partition 0: [ones(128) | b3r(512)]
    row_d = nc.dram_tensor("rowc", [1, 128 + DIM], F32R, kind="ExternalInput")
    # misc per-partition scalars: [b1a b1b b2a b2b b3c(4) tsc] = 9 cols
    msc_d = nc.dram_tensor("mscp", [128, 9], F32, kind="ExternalInput")
    y_d = nc.dram_tensor("y", [R, DIM], F32, kind="ExternalOutput")

    with tile.TileContext(nc) as tc:
        with (
            tc.tile_pool(name="const", bufs=1) as const,
            tc.tile_pool(name="work", bufs=2) as work,
            tc.tile_pool(name="psum", bufs=2, space="PSUM") as ps,
            tc.tile_pool(name="psuml", bufs=3, space="PSUM") as psl,
        ):
            sl = np.s_

            def load(name, dram, shape, dtype, nchunks, eng):
                """Allocate an SBUF tile and DMA it in `nchunks` column
                chunks (parallel DMA engines); issue from `eng`'s queue."""
                tl = const.tile(shape, dtype, tag=name)
                w = shape[1] // nchunks
                for k in range(nchunks):
                    eng.dma_start(tl[:, k * w:(k + 1) * w],
                                  dram[:, k * w:(k + 1) * w])
                return tl

            # Loads are split across the SP and GPSIMD queues (both idle)
            # so issue serialization (~0.65us per DMA instruction) overlaps;
            # need-ordered per queue: MLP gate first, loop constants, tail.
            SP, GP = nc.sync, nc.gpsimd
            xT = load("xT", xT_d, [128, 2 * R], F32R, 4, SP)
            w1 = load("w1", w1_d, [128, 2 * HID], F32R, 4, GP)
            w2 = load("w2", w2_d, [128, 2 * HID], F32R, 2, GP)
            msc = load("msc", msc_d, [128, 9], F32, 1, GP)
            row = load("row", row_d, [1, 128 + DIM], F32R, 1, GP)
            w3 = load("w3", w3_d, [128, 2 * DIM], F32R, 4, SP)
            hm = load("hm", h_d, [128, 2 * M], F32R, 2, SP)
            bts = load("bts", bts_d, [128, 2 * R], F32, 4, GP)
            at = load("at", at_d, [128, 4 * M], F32R, 4, SP)
            if VARIANT != "C":
                eye = load("eye", eye_d, [128, 128], F32R, 1, GP)
            na = load("na", na_d, [128, 2 * DIM], F32R, 2, GP)

            b1a, b1b = msc[:, 0:1], msc[0:72, 1:2]
            b2a, b2b = msc[:, 2:3], msc[0:72, 3:4]
            b3c = msc[:, 4:8]
            tsc = msc[:, 8:9]
            ones = row[0:1, 0:128]
            b3r = row[0:1, 128:128 + DIM]
            w2a, w2b = w2[:, 0:HID], w2[0:72, HID:2 * HID]
            w3a, w3b = w3[:, 0:DIM], w3[0:72, DIM:2 * DIM]

            mm = nc.tensor.matmul

            # ------------------------------------------------ MLP (transposed)
            # h1T = relu(W1.T @ xT + b1)   [200, R] in two partition chunks
            h1a = const.tile([128, R], F32R, tag="h1a")
            h1b = const.tile([72, R], F32R, tag="h1b")
            p = ps.tile([128, R], F32, tag="setup")
            mm(p[:], w1[:, 0:128], xT[:, 0:R], start=True, stop=False)
            mm(p[:], w1[:, HID:HID + 128], xT[:, R:2 * R], start=False, stop=True)
            nc.scalar.activation(h1a[:], p[:], AF.Relu, bias=b1a)
            p = ps.tile([72, R], F32, tag="setup")
            mm(p[:], w1[:, 128:HID], xT[:, 0:R], start=True, stop=False)
            mm(p[:], w1[:, HID + 128:2 * HID], xT[:, R:2 * R], start=False,
               stop=True)
            nc.vector.tensor_scalar(h1b[:], p[:], b1b, 0.0, op0=OP.add,
                                    op1=OP.max)

            # h2T = relu(W2.T @ h1T + b2)
            h2a = const.tile([128, R], F32R, tag="h2a")
            h2b = const.tile([72, R], F32R, tag="h2b")
            p = ps.tile([128, R], F32, tag="setup")
            mm(p[:], w2a[:, 0:128], h1a[:], start=True, stop=False)
            mm(p[:], w2b[:, 0:128], h1b[:], start=False, stop=True)
            nc.scalar.activation(h2a[:], p[:], AF.Relu, bias=b2a)
            p = ps.tile([72, R], F32, tag="setup")
            mm(p[:], w2a[:, 128:HID], h1a[:], start=True, stop=False)
            mm(p[:], w2b[:, 128:HID], h1b[:], start=False, stop=True)
            nc.vector.tensor_scalar(h2b[:], p[:], b2b, 0.0, op0=OP.add,
                                    op1=OP.max)

            # y0T = W3.T @ h2T + b3    [512, R] in 4 chunks
            y0T = const.tile([128, 4 * R], F32R, tag="y0T")
            for j in range(4):
                p = ps.tile([128, R], F32, tag="setup")
                mm(p[:], w3a[:, j * 128:(j + 1) * 128], h2a[:], start=True,
                   stop=False)
                mm(p[:], w3b[:, j * 128:(j + 1) * 128], h2b[:], start=False,
                   stop=True)
                if j % 2 == 0:
                    nc.scalar.activation(y0T[:, j * R:(j + 1) * R], p[:],
                                         AF.Identity, bias=b3c[:, j:j + 1])
                else:
                    nc.vector.tensor_scalar(y0T[:, j * R:(j + 1) * R], p[:],
                                            b3c[:, j:j + 1], None, op0=OP.add)

            # cT = t*(A @ y0.T) - t*b.T      [256, R] in 2 chunks
            cT = const.tile([128, 2 * R], F32R, tag="cT")
            for mj in range(2):
                p = ps.tile([128, R], F32, tag="setup")
                for dk in range(4):
                    mm(p[:], at[:, dk * M + mj * 128:dk * M + (mj + 1) * 128],
                       y0T[:, dk * R:(dk + 1) * R], start=(dk == 0),
                       stop=(dk == 3))
                nc.vector.scalar_tensor_tensor(
                    cT[:, mj * R:(mj + 1) * R], p[:], tsc,
                    bts[:, mj * R:(mj + 1) * R], op0=OP.mult, op1=OP.add)

            # ------------------------------------------------ projection loop
            # lam_1 = relu(c); halves on ACT / DVE (one sem each, like loop)
            AC = 276  # ACT/DVE column split (used by final-stage copies)
            lam = work.tile([128, 2 * R], F32R, tag="lam")
            nc.scalar.activation(lam[:, 0:R], cT[:, 0:R], AF.Relu)
            nc.vector.tensor_scalar(lam[:, R:2 * R], cT[:, R:2 * R], 0.0,
                                    None, op0=OP.max)

            if VARIANT == "C":
                for _ in range(n_iter - 1):
                    new = work.tile([128, 2 * R], F32R, tag="lam")
                    p0 = psl.tile([128, R], F32, tag="p0")
                    p1 = psl.tile([128, R], F32, tag="p1")
                    mm(p0[:], hm[:, 0:128], lam[:, 0:R], start=True,
                       stop=False)
                    mm(p0[:], hm[:, M:M + 128], lam[:, R:2 * R], start=False,
                       stop=True)
                    mm(p1[:], hm[:, 128:M], lam[:, 0:R], start=True,
                       stop=False)
                    mm(p1[:], hm[:, M + 128:2 * M], lam[:, R:2 * R],
                       start=False, stop=True)
                    nc.vector._custom_dve(relu_add, out=new[:, 0:R],
                                          in0=p0[:], in1=cT[:, 0:R])
                    nc.vector._custom_dve(relu_add, out=new[:, R:2 * R],
                                          in0=p1[:], in1=cT[:, R:2 * R])
                    lam = new
            else:
                # Software-pipelined: the eye-matmuls (c-add, group start)
                # for iteration i+1 are issued at the END of iteration i's
                # matmul stream, so the PE reaches each lam-wait late (all
                # waits evaluate already-satisfied -> no LDW exposure) and
                # the iteration boundary carries no stall.
                p0 = psl.tile([128, R], F32, tag="p0")
                p1 = psl.tile([128, R], F32, tag="p1")
                mm(p0[:], eye[:], cT[:, 0:R], start=True, stop=False)
                mm(p1[:], eye[:], cT[:, R:2 * R], start=True, stop=False)
                for it in range(n_iter - 1):
                    new = work.tile([128, 2 * R], F32R, tag="lam")
                    mm(p0[:], hm[:, 0:128], lam[:, 0:R], start=False,
                       stop=False)
                    mm(p1[:], hm[:, 128:M], lam[:, 0:R], start=False,
                       stop=False)
                    mm(p0[:], hm[:, M:M + 128], lam[:, R:2 * R], start=False,
                       stop=True)
                    mm(p1[:], hm[:, M + 128:2 * M], lam[:, R:2 * R],
                       start=False, stop=True)
                    if it < n_iter - 2:
                        n0 = psl.tile([128, R], F32, tag="p0")
                        n1 = psl.tile([128, R], F32, tag="p1")
                        mm(n0[:], eye[:], cT[:, 0:R], start=True, stop=False)
                        mm(n1[:], eye[:], cT[:, R:2 * R], start=True,
                           stop=False)
                    nc.scalar.activation(new[:, 0:R], p0[:], AF.Relu)
                    nc.vector.tensor_scalar(new[:, R:2 * R], p1[:], 0.0,
                                            None, op0=OP.max)
                    if it < n_iter - 2:
                        p0, p1 = n0, n1
                    lam = new

            # ------------------------------------------------ y = y0 - lam@A
            # row-major per row-tile: psum = h2.T@W3 + 1.b3 + lam.T@(-A)
            for rt in range(4):
                p = ps.tile([128, DIM], F32, tag="setup")
                mm(p[:], h2a[:, rt * 128:(rt + 1) * 128], w3a[:], start=True,
                   stop=False)
                mm(p[:], h2b[:, rt * 128:(rt + 1) * 128], w3b[:], start=False,
                   stop=False)
                mm(p[:], ones, b3r, start=False, stop=False)
                mm(p[:], lam[:, rt * 128:(rt + 1) * 128], na[:, 0:DIM],
                   start=False, stop=False)
                mm(p[:], lam[:, R + rt * 128:R + (rt + 1) * 128],
                   na[:, DIM:2 * DIM], start=False, stop=True)
                yt = work.tile([128, DIM], F32, tag="yout")
                # psum -> sbuf copy split across ACT and DVE for speed
                nc.scalar.copy(yt[:, 0:AC], p[:, 0:AC])
                nc.vector.tensor_copy(yt[:, AC:DIM], p[:, AC:DIM])
                # output DMAs on the idle SP/GPSIMD queues (2 chunks per
                # row-tile for transfer parallelism without issue pileup)
                for k, eng in ((0, nc.sync), (1, nc.gpsimd)):
                    eng.dma_start(
                        y_d[rt * 128:(rt + 1) * 128, k * 256:(k + 1) * 256],
                        yt[:, k * 256:(k + 1) * 256])

    nc.compile()
    return nc


def _pack_inputs(x, b, W1, b1, W2, b2, W3, b3, A):
    """Host-side packing into 128-partition dram layouts."""
    t = np.float32(1.0) / np.sum(A * A, dtype=np.float32)
    H = (np.eye(M, dtype=np.float32)
         - t * (A @ A.T).astype(np.float32)).astype(np.float32)

    def stack2(m):  # [256, K] -> [128, 2K]
        return np.ascontiguousarray(np.hstack([m[0:128], m[128:256]]))

    w1p = stack2(W1)                                   # [128, 400]
    w2p = np.zeros((128, 2 * HID), np.float32)         # [128, 400], padded
    w2p[:, 0:HID] = W2[0:128]
    w2p[0:72, HID:2 * HID] = W2[128:HID]
    w3p = np.zeros((128, 2 * DIM), np.float32)         # [128, 1024], padded
    w3p[:, 0:DIM] = W3[0:128]
    w3p[0:72, DIM:2 * DIM] = W3[128:HID]
    AT = np.ascontiguousarray(A.T)                     # [512, 256]
    atp = np.ascontiguousarray(
        np.hstack([AT[k * 128:(k + 1) * 128] for k in range(4)]))
    nap = stack2(np.ascontiguousarray(-A))             # [128, 1024]
    hmp = stack2(H)                                    # [128, 512]
    rowc = np.zeros((1, 128 + DIM), np.float32)
    rowc[0, 0:128] = 1.0
    rowc[0, 128:] = b3
    mscp = np.zeros((128, 9), np.float32)
    mscp[:, 0] = b1[0:128]
    mscp[0:72, 1] = b1[128:HID]
    mscp[:, 2] = b2[0:128]
    mscp[0:72, 3] = b2[128:HID]
    mscp[:, 4:8] = b3.reshape(4, 128).T
    mscp[:, 8] = t
    shared = {
        "w1p": w1p, "w2p": w2p, "w3p": w3p, "atp": atp, "nap": nap,
        "hmp": hmp, "rowc": rowc, "mscp": mscp,
        "eye": np.eye(128, dtype=np.float32),
    }
    return t, shared


def kernel(**inputs) -> np.ndarray:
    from concourse.bass_utils import run_bass_kernel_spmd

    x = np.asarray(inputs["x"], dtype=np.float32)
    b = np.asarray(inputs["b"], dtype=np.float32)
    W1 = np.asarray(inputs["W1"], dtype=np.float32)
    b1 = np.asarray(inputs["b1"], dtype=np.float32)
    W2 = np.asarray(inputs["W2"], dtype=np.float32)
    b2 = np.asarray(inputs["b2"], dtype=np.float32)
    W3 = np.asarray(inputs["W3"], dtype=np.float32)
    b3 = np.asarray(inputs["b3"], dtype=np.float32)
    A = np.asarray(inputs["A"], dtype=np.float32)
    n_iter = int(inputs.get("n_iter", 100))

    if n_iter not in _BUILD_CACHE:
        _BUILD_CACHE[n_iter] = _build(n_iter)
    nc = _BUILD_CACHE[n_iter]

    t, shared = _pack_inputs(x, b, W1, b1, W2, b2, W3, b3, A)
    in_maps = []
    for c in range(NCORES):
        r0, r1 = c * R, (c + 1) * R
        m = dict(shared)
        xT = np.ascontiguousarray(x[r0:r1].T)          # [256, R]
        m["xTp"] = np.ascontiguousarray(np.hstack([xT[0:128], xT[128:256]]))
        btsT = (-t) * b[r0:r1].T                       # [256, R]
        m["btsp"] = np.ascontiguousarray(
            np.hstack([btsT[0:128], btsT[128:256]]).astype(np.float32))
        in_maps.append(m)

    run_kwargs = {}
    if TRACE:
        _install_ntff_hook()
        tc_env = os.environ.get("TRACE_CORES", "0")
        cores = list(range(NCORES)) if tc_env == "all" else [
            int(v) for v in tc_env.split(",")]
        run_kwargs = dict(trace=True, trace_cores=cores)

    res = run_bass_kernel_spmd(nc, in_maps, list(range(NCORES)), **run_kwargs)
    if TRACE:
        LAST["exec_time_ns"] = res.exec_time_ns
        LAST["mean_exec_time_ns"] = res.mean_exec_time_ns
        if res.instructions_and_trace is not None:
            LAST["trace"] = res.instructions_and_trace[1]
    y = np.concatenate([res.results[c]["y"] for c in range(NCORES)], axis=0)
    return y.astype(np.float32)


def _install_ntff_hook():
    """Register the antenv.axon_hooks NTFF profile hook (absent from this
    image) so run_bass_kernel_spmd(trace=True) can neuron-profile under
    axon.  Only invoked on TRACE runs (test harness), never when grading."""
    import contextlib
    import ctypes
    import types

    if "antenv.axon_hooks" in sys.modules:
        return
    so_path = "/opt/axon/libaxon_pjrt.so"
    lib = ctypes.CDLL(so_path)
    if not hasattr(lib, "axon_start_nrt_profile"):
        return
    lib.axon_start_nrt_profile.argtypes = [ctypes.POINTER(ctypes.c_int64),
                                           ctypes.c_size_t]
    lib.axon_start_nrt_profile.restype = ctypes.c_int64
    lib.axon_stop_nrt_profile.argtypes = [ctypes.c_char_p]
    lib.axon_stop_nrt_profile.restype = ctypes.c_int64

    @contextlib.contextmanager
    def _hook(output_dir, device_ids):
        import jax
        jax.devices()
        if device_ids:
            ids = (ctypes.c_int64 * len(device_ids))(*device_ids)
            rc = lib.axon_start_nrt_profile(ids, len(device_ids))
        else:
            rc = lib.axon_start_nrt_profile(None, 0)
        if rc != 0:
            raise RuntimeError(f"axon_start_nrt_profile rc={rc}")
        try:
            yield
        finally:
            n = lib.axon_stop_nrt_profile(str(output_dir).encode())
            if n <= 0:
                print(f"ntff profile: {n} files written", file=sys.stderr)

    mod = types.ModuleType("antenv.axon_hooks")
    _state = {"hook": _hook}
    mod.set_axon_ntff_profile_hook = lambda h: _state.__setitem__("hook", h)
    mod.get_axon_ntff_profile_hook = lambda: _state["hook"]
    sys.modules["antenv.axon_hooks"] = mod
    import antenv
    antenv.axon_hooks = mod
    from concourse import bass_utils
    bass_utils.upload_artifacts = lambda tmpdir: f"file://{tmpdir}"


if __name__ == "__main__":
    # quick self-driven smoke: random inputs, compare against numpy pipeline
    rng = np.random.default_rng(0)
    ins = {
        "x": rng.standard_normal((B, IN_DIM)).astype(np.float32),
        "b": (rng.random((B, M)) + 1.0).astype(np.float32),
        "W1": (rng.standard_normal((IN_DIM, HID)) / 16.0).astype(np.float32),
        "b1": np.zeros(HID, np.float32),
        "W2": (rng.standard_normal((HID, HID)) / 14.14).astype(np.float32),
        "b2": np.zeros(HID, np.float32),
        "W3": (rng.standard_normal((HID, DIM)) / 14.14).astype(np.float32),
        "b3": np.zeros(DIM, np.float32),
        "A": (rng.standard_normal((M, DIM)) / 22.6).astype(np.float32),
        "step": 0,
        "n_iter": 100,
    }
    y = kernel(**ins)

    xx, bb, AA = ins["x"], ins["b"], ins["A"]
    h = np.maximum(xx @ ins["W1"] + ins["b1"], 0).astype(np.float32)
    h = np.maximum(h @ ins["W2"] + ins["b2"], 0).astype(np.float32)
    y0 = (h @ ins["W3"] + ins["b3"]).astype(np.float32)
    t = np.float32(1.0) / np.sum(AA * AA, dtype=np.float32)
    lam = np.zeros_like(bb)
    for _ in range(100):
        yy = (y0 - lam @ AA).astype(np.float32)
        lam = np.maximum(lam + t * ((yy @ AA.T).astype(np.float32) - bb), 0)
    yref = y0 - (lam @ AA).astype(np.float32)
    rel = np.linalg.norm(y - yref) / np.linalg.norm(yref)
    print("self-test rel err:", rel)


# revision 17
# speedup vs baseline: 1.0113x; 1.0113x over previous
"""Trainium2 Bass kernel for HardConstrainedMLP (MLP + 100-iter dual projected
gradient projection onto {y : Ay <= b}).

Math rewrite (verified to 4.6e-7 vs reference):
    y0 = MLP(x)
    t  = 1/||A||_F^2 ; G = A@A.T ; H = I - t*G ; c = t*(y0@A.T - b)
    lam_{i+1} = relu(lam_i @ H + c)        (100 iters, lam_0 = 0)
    y = y0 - lam_100 @ A

On-device layout is transposed (feature-major) so the per-iteration matmul
chain needs no transposes.  Matmuls use float32r (full PE rate at >=256
moving columns).

Inner loop (variant A, default): per iteration two PSUM tiles accumulate
eye@c + H-block matmuls (6 matmuls, 512-wide moving operands = full f32r
rate); relu consumers run unsplit on ACT / DVE (one semaphore per lam
half -- splitting them measurably loses to PE-queue wait-eval cost), and
the next iteration's eye-matmuls are issued at the end of each iteration
so the loop boundary carries no extra stall.  Measured ~1760 ns/iter,
which matches the order-LP floor for this shape (psum-fire -> relu ->
consume gap of ~800 ns against 6x236 ns of PE work).

Variant C (KERNEL_VARIANT=C) keeps a custom fused DVE op relu(psum + c)
with 4 matmuls/iter; measured slower (DVE-bound at ~1880 ns/iter).

Data-parallel over batch: 4096 rows -> 8 cores x 512 rows.
"""

import os
import sys

sys.path.insert(0, "/opt/trn_rl_repo")

import numpy as np

B, IN_DIM, HID, DIM, M = 4096, 256, 200, 512, 256
NCORES = 8
R = B // NCORES  # rows per core

VARIANT = os.environ.get("KERNEL_VARIANT", "A")  # "A" eye-mm | "C" fused-dve
TRACE = False  # set by test.py before calling kernel() to profile the run
LAST = {}      # {"exec_time_ns": int, "trace": path} after a TRACE run

_BUILD_CACHE = {}
_RELU_ADD = None


def _get_relu_add_op():
    """Register (once) a custom DVE op computing out = relu(in0 + in1).

    Uses the stock custom-DVE plumbing in concourse.dve_ops: we assign the
    next free byte-36 opcode row and append to OPS so the per-NEFF table
    generation picks it up."""
    global _RELU_ADD
    if _RELU_ADD is not None:
        return _RELU_ADD
    import concourse.dve_ops as dops
    from concourse.dve_spec import Spec, Src0, Src1, lower, relu, _has_src1
    from concourse.dve_table_gen import dve_ver_for
    from concourse.dve_uop import DveOpSpec

    name = "RELU_ADD_ANT"

    def _ref(in0, in1, s0, s1, imm2):
        x = in0.astype(np.float32) + in1.astype(np.float32)
        x = np.nan_to_num(x, nan=0.0, posinf=np.inf, neginf=-np.inf)
        return np.maximum(x, 0).astype(np.float32)

    spec = Spec(body=relu(Src0 + Src1), reference=_ref)
    if name not in dops._SUB_OPCODE_FOR_NAME:
        row = max(dops._SUB_OPCODE_FOR_NAME.values()) + 1
        assert row < 0x20, "custom-DVE opcode rows exhausted"
        dops._SUB_OPCODE_FOR_NAME[name] = row
        ver = dve_ver_for("TRN2")
        uops = lower(spec, ver=ver)
        sha = DveOpSpec(name=name, opcode=row, uops=uops,
                        rd1_en=_has_src1(spec)).sha(ver)
        op = dops.DveOp(name, spec, subdim=False, uops_sha={ver: sha})
        dops.OPS.append(op)
        dops.CUSTOM_DVE_SPECS[name] = spec
    else:
        op = next(o for o in dops.OPS if o.name == name)
    _RELU_ADD = op
    return op


def _build(n_iter: int):
    import concourse.mybir as mybir
    import concourse.tile as tile
    from concourse import bacc

    F32 = mybir.dt.float32
    F32R = mybir.dt.float32r
    AF = mybir.ActivationFunctionType
    OP = mybir.AluOpType

    relu_add = _get_relu_add_op() if VARIANT == "C" else None

    nc = bacc.Bacc("TRN2", target_bir_lowering=False, debug=False,
                   num_devices=NCORES)

    # ---- per-core inputs (f32r dram = raw fp32 bytes used as matmul operands)
    # All large tensors are host-packed to 128 partitions so each is a single
    # dram tensor; DMAs are column-chunked for transfer parallelism.
    xT_d = nc.dram_tensor("xTp", [128, 2 * R], F32R, kind="ExternalInput")
    bts_d = nc.dram_tensor("btsp", [128, 2 * R], F32, kind="ExternalInput")
    # ---- replicated weights / constants (packed)
    w1_d = nc.dram_tensor("w1p", [128, 2 * HID], F32R, kind="ExternalInput")
    w2_d = nc.dram_tensor("w2p", [128, 2 * HID], F32R, kind="ExternalInput")
    w3_d = nc.dram_tensor("w3p", [128, 2 * DIM], F32R, kind="ExternalInput")
    at_d = nc.dram_tensor("atp", [128, 4 * M], F32R, kind="ExternalInput")
    na_d = nc.dram_tensor("nap", [128, 2 * DIM], F32R, kind="ExternalInput")
    h_d = nc.dram_tensor("hmp", [128, 2 * M], F32R, kind="ExternalInput")
    eye_d = nc.dram_tensor("eye", [128, 128], F32R, kind="ExternalInput")
    # row constants on partition 0: [ones(128) | b3r(512)]
    row_d = nc.dram_tensor("rowc", [1, 128 + DIM], F32R, kind="ExternalInput")
    # misc per-partition scalars: [b1a b1b b2a b2b b3c(4) tsc] = 9 cols
    msc_d = nc.dram_tensor("mscp", [128, 9], F32, kind="ExternalInput")
    y_d = nc.dram_tensor("y", [R, DIM], F32, kind="ExternalOutput")

    with tile.TileContext(nc) as tc:
        with (
            tc.tile_pool(name="const", bufs=1) as const,
            tc.tile_pool(name="work", bufs=2) as work,
            tc.tile_pool(name="psum", bufs=2, space="PSUM") as ps,
            tc.tile_pool(name="psuml", bufs=3, space="PSUM") as psl,
        ):
            sl = np.s_

            def load(name, dram, shape, dtype, nchunks, eng):
                """Allocate an SBUF tile and DMA it in `nchunks` column
                chunks (parallel DMA engines); issue from `eng`'s queue."""
                tl = const.tile(shape, dtype, tag=name)
                w = shape[1] // nchunks
                for k in range(nchunks):
                    eng.dma_start(tl[:, k * w:(k + 1) * w],
                                  dram[:, k * w:(k + 1) * w])
                return tl

            # Loads are split across the SP and GPSIMD queues (both idle)
            # so issue serialization (~0.65us per DMA instruction) overlaps;
            # need-ordered per queue: MLP gate first, loop constants, tail.
            # DMA completion semaphores are per-queue monotonic counters, so
            # a wait for tensor k implies waiting for every earlier transfer
            # on that queue: small early-needed tensors (msc feeds the very
            # first relu's bias) must be issued FIRST; tail-only tensors
            # (row, na) last.
            SP, GP = nc.sync, nc.gpsimd
            msc = load("msc", msc_d, [128, 9], F32, 1, GP)
            xT = load("xT", xT_d, [128, 2 * R], F32R, 4, SP)
            w1 = load("w1", w1_d, [128, 2 * HID], F32R, 4, GP)
            w2 = load("w2", w2_d, [128, 2 * HID], F32R, 2, GP)
            w3 = load("w3", w3_d, [128, 2 * DIM], F32R, 4, SP)
            at = load("at", at_d, [128, 4 * M], F32R, 4, SP)
            hm = load("hm", h_d, [128, 2 * M], F32R, 2, SP)
            bts = load("bts", bts_d, [128, 2 * R], F32, 4, GP)
            if VARIANT != "C":
                eye = load("eye", eye_d, [128, 128], F32R, 1, GP)
            na = load("na", na_d, [128, 2 * DIM], F32R, 2, GP)
            row = load("row", row_d, [1, 128 + DIM], F32R, 1, GP)

            b1a, b1b = msc[:, 0:1], msc[0:72, 1:2]
            b2a, b2b = msc[:, 2:3], msc[0:72, 3:4]
            b3c = msc[:, 4:8]
            tsc = msc[:, 8:9]
            ones = row[0:1, 0:128]
            b3r = row[0:1, 128:128 + DIM]
            w2a, w2b = w2[:, 0:HID], w2[0:72, HID:2 * HID]
            w3a, w3b = w3[:, 0:DIM], w3[0:72, DIM:2 * DIM]

            mm = nc.tensor.matmul

            # ------------------------------------------------ MLP (transposed)
            # h1T = relu(W1.T @ xT + b1)   [200, R] in two partition chunks
            h1a = const.tile([128, R], F32R, tag="h1a")
            h1b = const.tile([72, R], F32R, tag="h1b")
            p = ps.tile([128, R], F32, tag="setup")
            mm(p[:], w1[:, 0:128], xT[:, 0:R], start=True, stop=False)
            mm(p[:], w1[:, HID:HID + 128], xT[:, R:2 * R], start=False, stop=True)
            nc.scalar.activation(h1a[:], p[:], AF.Relu, bias=b1a)
            p = ps.tile([72, R], F32, tag="setup")
            mm(p[:], w1[:, 128:HID], xT[:, 0:R], start=True, stop=False)
            mm(p[:], w1[:, HID + 128:2 * HID], xT[:, R:2 * R], start=False,
               stop=True)
            nc.vector.tensor_scalar(h1b[:], p[:], b1b, 0.0, op0=OP.add,
                                    op1=OP.max)

            # h2T = relu(W2.T @ h1T + b2)
            h2a = const.tile([128, R], F32R, tag="h2a")
            h2b = const.tile([72, R], F32R, tag="h2b")
            p = ps.tile([128, R], F32, tag="setup")
            mm(p[:], w2a[:, 0:128], h1a[:], start=True, stop=False)
            mm(p[:], w2b[:, 0:128], h1b[:], start=False, stop=True)
            nc.scalar.activation(h2a[:], p[:], AF.Relu, bias=b2a)
            p = ps.tile([72, R], F32, tag="setup")
            mm(p[:], w2a[:, 128:HID], h1a[:], start=True, stop=False)
            mm(p[:], w2b[:, 128:HID], h1b[:], start=False, stop=True)
            nc.vector.tensor_scalar(h2b[:], p[:], b2b, 0.0, op0=OP.add,
                                    op1=OP.max)

            # y0T = W3.T @ h2T + b3    [512, R] in 4 chunks
            y0T = const.tile([128, 4 * R], F32R, tag="y0T")
            for j in range(4):
                p = ps.tile([128, R], F32, tag="setup")
                mm(p[:], w3a[:, j * 128:(j + 1) * 128], h2a[:], start=True,
                   stop=False)
                mm(p[:], w3b[:, j * 128:(j + 1) * 128], h2b[:], start=False,
                   stop=True)
                if j % 2 == 0:
                    nc.scalar.activation(y0T[:, j * R:(j + 1) * R], p[:],
                                         AF.Identity, bias=b3c[:, j:j + 1])
                else:
                    nc.vector.tensor_scalar(y0T[:, j * R:(j + 1) * R], p[:],
                                            b3c[:, j:j + 1], None, op0=OP.add)

            # cT = t*(A @ y0.T) - t*b.T      [256, R] in 2 chunks
            cT = const.tile([128, 2 * R], F32R, tag="cT")
            for mj in range(2):
                p = ps.tile([128, R], F32, tag="setup")
                for dk in range(4):
                    mm(p[:], at[:, dk * M + mj * 128:dk * M + (mj + 1) * 128],
                       y0T[:, dk * R:(dk + 1) * R], start=(dk == 0),
                       stop=(dk == 3))
                nc.vector.scalar_tensor_tensor(
                    cT[:, mj * R:(mj + 1) * R], p[:], tsc,
                    bts[:, mj * R:(mj + 1) * R], op0=OP.mult, op1=OP.add)

            # ------------------------------------------------ projection loop
            # lam_1 = relu(c); halves on ACT / DVE (one sem each, like loop)
            AC = 276  # ACT/DVE column split (used by final-stage copies)
            lam = work.tile([128, 2 * R], F32R, tag="lam")
            nc.scalar.activation(lam[:, 0:R], cT[:, 0:R], AF.Relu)
            nc.vector.tensor_scalar(lam[:, R:2 * R], cT[:, R:2 * R], 0.0,
                                    None, op0=OP.max)

            if VARIANT == "C":
                for _ in range(n_iter - 1):
                    new = work.tile([128, 2 * R], F32R, tag="lam")
                    p0 = psl.tile([128, R], F32, tag="p0")
                    p1 = psl.tile([128, R], F32, tag="p1")
                    mm(p0[:], hm[:, 0:128], lam[:, 0:R], start=True,
                       stop=False)
                    mm(p0[:], hm[:, M:M + 128], lam[:, R:2 * R], start=False,
                       stop=True)
                    mm(p1[:], hm[:, 128:M], lam[:, 0:R], start=True,
                       stop=False)
                    mm(p1[:], hm[:, M + 128:2 * M], lam[:, R:2 * R],
                       start=False, stop=True)
                    nc.vector._custom_dve(relu_add, out=new[:, 0:R],
                                          in0=p0[:], in1=cT[:, 0:R])
                    nc.vector._custom_dve(relu_add, out=new[:, R:2 * R],
                                          in0=p1[:], in1=cT[:, R:2 * R])
                    lam = new
            else:
                # Software-pipelined: the eye-matmuls (c-add, group start)
                # for iteration i+1 are issued at the END of iteration i's
                # matmul stream, so the PE reaches each lam-wait late (all
                # waits evaluate already-satisfied -> no LDW exposure) and
                # the iteration boundary carries no stall.
                #
                # The final stage's lam-independent part (y0 row-major =
                # h2.T@W3 + b3) is computed INSIDE the loop, one instruction
                # per iteration, tucked into the PE's ~240ns boundary idle
                # and the ACT/DVE idle windows; the post-loop tail then only
                # needs 3 matmuls per row-tile.
                y0r = [const.tile([128, DIM], F32R, tag=f"y0r{rt}",
                                  name=f"y0r{rt}") for rt in range(4)]
                ppre = {}

                def _mk_mm(rt, j):
                    def f():
                        if j == 0:
                            ppre[rt] = ps.tile([128, DIM], F32, tag="setup",
                                               name=f"y0rp{rt}")
                            mm(ppre[rt][:], h2a[:, rt * 128:(rt + 1) * 128],
                               w3a[:], start=True, stop=False)
                        elif j == 1:
                            mm(ppre[rt][:], h2b[:, rt * 128:(rt + 1) * 128],
                               w3b[:], start=False, stop=False)
                        else:
                            mm(ppre[rt][:], ones, b3r, start=False, stop=True)
                    return f

                def _mk_copy(rt):
                    def f():
                        p = ppre[rt]
                        nc.scalar.copy(y0r[rt][:, 0:AC], p[:, 0:AC])
                        nc.vector.tensor_copy(y0r[rt][:, AC:DIM],
                                              p[:, AC:DIM])
                    return f

                sched = []
                for rt in range(4):
                    sched += [("pe", _mk_mm(rt, 0)), ("pe", _mk_mm(rt, 1)),
                              ("pe", _mk_mm(rt, 2)), ("ew", _mk_copy(rt))]
                START = 2
                if n_iter - 1 < START + len(sched) + 1:
                    for _, f in sched:
                        f()
                    sched = []

                p0 = psl.tile([128, R], F32, tag="p0")
                p1 = psl.tile([128, R], F32, tag="p1")
                mm(p0[:], eye[:], cT[:, 0:R], start=True, stop=False)
                mm(p1[:], eye[:], cT[:, R:2 * R], start=True, stop=False)
                for it in range(n_iter - 1):
                    new = work.tile([128, 2 * R], F32R, tag="lam")
                    mm(p0[:], hm[:, 0:128], lam[:, 0:R], start=False,
                       stop=False)
                    mm(p1[:], hm[:, 128:M], lam[:, 0:R], start=False,
                       stop=False)
                    mm(p0[:], hm[:, M:M + 128], lam[:, R:2 * R], start=False,
                       stop=True)
                    mm(p1[:], hm[:, M + 128:2 * M], lam[:, R:2 * R],
                       start=False, stop=True)
                    if it < n_iter - 2:
                        n0 = psl.tile([128, R], F32, tag="p0")
                        n1 = psl.tile([128, R], F32, tag="p1")
                        mm(n0[:], eye[:], cT[:, 0:R], start=True, stop=False)
                        mm(n1[:], eye[:], cT[:, R:2 * R], start=True,
                           stop=False)
                    k = it - START
                    if 0 <= k < len(sched) and sched[k][0] == "pe":
                        sched[k][1]()
                    nc.scalar.activation(new[:, 0:R], p0[:], AF.Relu)
                    nc.vector.tensor_scalar(new[:, R:2 * R], p1[:], 0.0,
                                            None, op0=OP.max)
                    if 0 <= k < len(sched) and sched[k][0] == "ew":
                        sched[k][1]()
                    if it < n_iter - 2:
                        p0, p1 = n0, n1
                    lam = new

            # ------------------------------------------------ y = y0 - lam@A
            # row-major per row-tile: psum = y0r (precomputed) + lam.T@(-A)
            for rt in range(4):
                p = ps.tile([128, DIM], F32, tag="setup")
                if VARIANT == "C":
                    mm(p[:], h2a[:, rt * 128:(rt + 1) * 128], w3a[:],
                       start=True, stop=False)
                    mm(p[:], h2b[:, rt * 128:(rt + 1) * 128], w3b[:],
                       start=False, stop=False)
                    mm(p[:], ones, b3r, start=False, stop=False)
                else:
                    mm(p[:], eye[:], y0r[rt][:], start=True, stop=False)
                mm(p[:], lam[:, rt * 128:(rt + 1) * 128], na[:, 0:DIM],
                   start=False, stop=False)
                mm(p[:], lam[:, R + rt * 128:R + (rt + 1) * 128],
                   na[:, DIM:2 * DIM], start=False, stop=True)
                yt = work.tile([128, DIM], F32, tag="yout")
                # psum -> sbuf copy split across ACT and DVE for speed
                nc.scalar.copy(yt[:, 0:AC], p[:, 0:AC])
                nc.vector.tensor_copy(yt[:, AC:DIM], p[:, AC:DIM])
                # output DMAs on the idle SP/GPSIMD queues (2 chunks per
                # row-tile for transfer parallelism without issue pileup)
                for k, eng in ((0, nc.sync), (1, nc.gpsimd)):
                    eng.dma_start(
                        y_d[rt * 128:(rt + 1) * 128, k * 256:(k + 1) * 256],
                        yt[:, k * 256:(k + 1) * 256])

    nc.compile()
    return nc


def _pack_inputs(x, b, W1, b1, W2, b2, W3, b3, A):
    """Host-side packing into 128-partition dram layouts."""
    t = np.float32(1.0) / np.sum(A * A, dtype=np.float32)
    H = (np.eye(M, dtype=np.float32)
         - t * (A @ A.T).astype(np.float32)).astype(np.float32)

    def stack2(m):  # [256, K] -> [128, 2K]
        return np.ascontiguousarray(np.hstack([m[0:128], m[128:256]]))

    w1p = stack2(W1)                                   # [128, 400]
    w2p = np.zeros((128, 2 * HID), np.float32)         # [128, 400], padded
    w2p[:, 0:HID] = W2[0:128]
    w2p[0:72, HID:2 * HID] = W2[128:HID]
    w3p = np.zeros((128, 2 * DIM), np.float32)         # [128, 1024], padded
    w3p[:, 0:DIM] = W3[0:128]
    w3p[0:72, DIM:2 * DIM] = W3[128:HID]
    AT = np.ascontiguousarray(A.T)                     # [512, 256]
    atp = np.ascontiguousarray(
        np.hstack([AT[k * 128:(k + 1) * 128] for k in range(4)]))
    nap = stack2(np.ascontiguousarray(-A))             # [128, 1024]
    hmp = stack2(H)                                    # [128, 512]
    rowc = np.zeros((1, 128 + DIM), np.float32)
    rowc[0, 0:128] = 1.0
    rowc[0, 128:] = b3
    mscp = np.zeros((128, 9), np.float32)
    mscp[:, 0] = b1[0:128]
    mscp[0:72, 1] = b1[128:HID]
    mscp[:, 2] = b2[0:128]
    mscp[0:72, 3] = b2[128:HID]
    mscp[:, 4:8] = b3.reshape(4, 128).T
    mscp[:, 8] = t
    shared = {
        "w1p": w1p, "w2p": w2p, "w3p": w3p, "atp": atp, "nap": nap,
        "hmp": hmp, "rowc": rowc, "mscp": mscp,
        "eye": np.eye(128, dtype=np.float32),
    }
    return t, shared


def kernel(**inputs) -> np.ndarray:
    from concourse.bass_utils import run_bass_kernel_spmd

    x = np.asarray(inputs["x"], dtype=np.float32)
    b = np.asarray(inputs["b"], dtype=np.float32)
    W1 = np.asarray(inputs["W1"], dtype=np.float32)
    b1 = np.asarray(inputs["b1"], dtype=np.float32)
    W2 = np.asarray(inputs["W2"], dtype=np.float32)
    b2 = np.asarray(inputs["b2"], dtype=np.float32)
    W3 = np.asarray(inputs["W3"], dtype=np.float32)
    b3 = np.asarray(inputs["b3"], dtype=np.float32)
    A = np.asarray(inputs["A"], dtype=np.float32)
    n_iter = int(inputs.get("n_iter", 100))

    if n_iter not in _BUILD_CACHE:
        _BUILD_CACHE[n_iter] = _build(n_iter)
    nc = _BUILD_CACHE[n_iter]

    t, shared = _pack_inputs(x, b, W1, b1, W2, b2, W3, b3, A)
    in_maps = []
    for c in range(NCORES):
        r0, r1 = c * R, (c + 1) * R
        m = dict(shared)
        xT = np.ascontiguousarray(x[r0:r1].T)          # [256, R]
        m["xTp"] = np.ascontiguousarray(np.hstack([xT[0:128], xT[128:256]]))
        btsT = (-t) * b[r0:r1].T                       # [256, R]
        m["btsp"] = np.ascontiguousarray(
            np.hstack([btsT[0:128], btsT[128:256]]).astype(np.float32))
        in_maps.append(m)

    run_kwargs = {}
    if TRACE:
        _install_ntff_hook()
        tc_env = os.environ.get("TRACE_CORES", "0")
        cores = list(range(NCORES)) if tc_env == "all" else [
            int(v) for v in tc_env.split(",")]
        run_kwargs = dict(trace=True, trace_cores=cores)

    res = run_bass_kernel_spmd(nc, in_maps, list(range(NCORES)), **run_kwargs)
    if TRACE:
        LAST["exec_time_ns"] = res.exec_time_ns
        LAST["mean_exec_time_ns"] = res.mean_exec_time_ns
        if res.instructions_and_trace is not None:
            LAST["trace"] = res.instructions_and_trace[1]
    y = np.concatenate([res.results[c]["y"] for c in range(NCORES)], axis=0)
    return y.astype(np.float32)


def _install_ntff_hook():
    """Register the antenv.axon_hooks NTFF profile hook (absent from this
    image) so run_bass_kernel_spmd(trace=True) can neuron-profile under
    axon.  Only invoked on TRACE runs (test harness), never when grading."""
    import contextlib
    import ctypes
    import types

    if "antenv.axon_hooks" in sys.modules:
        return
    so_path = "/opt/axon/libaxon_pjrt.so"
    lib = ctypes.CDLL(so_path)
    if not hasattr(lib, "axon_start_nrt_profile"):
        return
    lib.axon_start_nrt_profile.argtypes = [ctypes.POINTER(ctypes.c_int64),
                                           ctypes.c_size_t]
    lib.axon_start_nrt_profile.restype = ctypes.c_int64
    lib.axon_stop_nrt_profile.argtypes = [ctypes.c_char_p]
    lib.axon_stop_nrt_profile.restype = ctypes.c_int64

    @contextlib.contextmanager
    def _hook(output_dir, device_ids):
        import jax
        jax.devices()
        if device_ids:
            ids = (ctypes.c_int64 * len(device_ids))(*device_ids)
            rc = lib.axon_start_nrt_profile(ids, len(device_ids))
        else:
            rc = lib.axon_start_nrt_profile(None, 0)
        if rc != 0:
            raise RuntimeError(f"axon_start_nrt_profile rc={rc}")
        try:
            yield
        finally:
            n = lib.axon_stop_nrt_profile(str(output_dir).encode())
            if n <= 0:
                print(f"ntff profile: {n} files written", file=sys.stderr)

    mod = types.ModuleType("antenv.axon_hooks")
    _state = {"hook": _hook}
    mod.set_axon_ntff_profile_hook = lambda h: _state.__setitem__("hook", h)
    mod.get_axon_ntff_profile_hook = lambda: _state["hook"]
    sys.modules["antenv.axon_hooks"] = mod
    import antenv
    antenv.axon_hooks = mod
    from concourse import bass_utils
    bass_utils.upload_artifacts = lambda tmpdir: f"file://{tmpdir}"


if __name__ == "__main__":
    # quick self-driven smoke: random inputs, compare against numpy pipeline
    rng = np.random.default_rng(0)
    ins = {
        "x": rng.standard_normal((B, IN_DIM)).astype(np.float32),
        "b": (rng.random((B, M)) + 1.0).astype(np.float32),
        "W1": (rng.standard_normal((IN_DIM, HID)) / 16.0).astype(np.float32),
        "b1": np.zeros(HID, np.float32),
        "W2": (rng.standard_normal((HID, HID)) / 14.14).astype(np.float32),
        "b2": np.zeros(HID, np.float32),
        "W3": (rng.standard_normal((HID, DIM)) / 14.14).astype(np.float32),
        "b3": np.zeros(DIM, np.float32),
        "A": (rng.standard_normal((M, DIM)) / 22.6).astype(np.float32),
        "step": 0,
        "n_iter": 100,
    }
    y = kernel(**ins)

    xx, bb, AA = ins["x"], ins["b"], ins["A"]
    h = np.maximum(xx @ ins["W1"] + ins["b1"], 0).astype(np.float32)
    h = np.maximum(h @ ins["W2"] + ins["b2"], 0).astype(np.float32)
    y0 = (h @ ins["W3"] + ins["b3"]).astype(np.float32)
    t = np.float32(1.0) / np.sum(AA * AA, dtype=np.float32)
    lam = np.zeros_like(bb)
    for _ in range(100):
        yy = (y0 - lam @ AA).astype(np.float32)
        lam = np.maximum(lam + t * ((yy @ AA.T).astype(np.float32) - bb), 0)
    yref = y0 - (lam @ AA).astype(np.float32)
    rel = np.linalg.norm(y - yref) / np.linalg.norm(yref)
    print("self-test rel err:", rel)
